# revision 3
# baseline (speedup 1.0000x reference)
"""Trainium2 Bass kernel for nn_MultiHeadAttention_377957122345.

B=16, T=512, C=1024, H=16, D=64.  Data-parallel over batch: each of the
8 NeuronCores computes attention for 2 sequences; no collectives.

Per-core device program (SPMD, identical on all cores):
  - inputs staged on host as transposed layouts: xT [C, 1024] (c_in on
    partitions), W^T [c_in, c_out] for all four projections, and
    rel_pos_bias with the causal mask folded in (-1e30 above diagonal).
  - all matmuls in bf16 with fp32 PSUM accumulation; softmax in fp32.
  - Q/K projections produce q^T/k^T (head_dim on partitions); V and the
    final output are produced in natural [t, c] layout.
  - causal block skipping: for query block i only key blocks j <= i are
    computed, masked entries never touched.
  - scores S = q^T.T @ k^T land in PSUM [128 t, w s]; bias added in
    PSUM; exp on ScalarE with row-sum accumulation; P normalized and
    transposed 128x128 via TensorE so the AV matmul can contract over s.
"""

import numpy as np

B, T, C, H = 16, 512, 1024, 16
D = C // H  # 64
N_CORES = 8
B_LOC = B // N_CORES  # 2 sequences per core
NT = B_LOC * T  # 1024 tokens per core
P = 128
KO = C // P  # 8 contraction subtiles
TB = T // P  # 4 query blocks per sequence
NEG = -1e30

_CACHE = {}


def _split_big_waits(nc, mybir, limit=1):
    # This walrus build rejects instructions whose sync_info.on_wait
    # exceeds its slot count (the Tile end-of-kernel Drain trips it).
    # Move excess waits onto dedicated same-engine NOPs placed directly
    # before the instruction; the engine stalls on those first, so the
    # semantics are unchanged.
    f = nc.m.functions[0]
    for bb in f.blocks:
        out = []
        changed = False
        for inst in bb.instructions:
            si = getattr(inst, "sync_info", None)
            waits = list(si.on_wait) if si is not None and si.on_wait else []
            if len(waits) > limit:
                changed = True
                head, tail = waits[:-limit], waits[-limit:]
                for k in range(0, len(head), limit):
                    out.append(
                        mybir.InstNoOp(
                            name=f"I-{nc.next_id()}",
                            sync_info=mybir.SyncInfo(
                                on_wait=head[k : k + limit], on_update=[]
                            ),
                            bass_nofuse=True,
                            engine=inst.engine,
                        )
                    )
                si.on_wait = tail
            out.append(inst)
        if changed:
            bb.instructions = out
    return nc


def build_program(split_waits=True):
    import concourse.bass as bass
    import concourse.mybir as mybir
    import concourse.tile as tile
    from concourse.masks import make_identity

    fp32 = mybir.dt.float32
    bf16 = mybir.dt.bfloat16
    Act = mybir.ActivationFunctionType

    nc = bass.Bass()
    xT = nc.dram_tensor("xT", [C, NT], fp32, kind="ExternalInput")
    wqT = nc.dram_tensor("wqT", [C, C], fp32, kind="ExternalInput")
    wkT = nc.dram_tensor("wkT", [C, C], fp32, kind="ExternalInput")
    wvT = nc.dram_tensor("wvT", [C, C], fp32, kind="ExternalInput")
    woT = nc.dram_tensor("woT", [C, C], fp32, kind="ExternalInput")
    bqd = nc.dram_tensor("bq", [C], fp32, kind="ExternalInput")
    bkd = nc.dram_tensor("bk", [C], fp32, kind="ExternalInput")
    bvd = nc.dram_tensor("bv", [C], fp32, kind="ExternalInput")
    bod = nc.dram_tensor("bo", [C], fp32, kind="ExternalInput")
    biasm = nc.dram_tensor("biasm", [H, T, T], fp32, kind="ExternalInput")
    y = nc.dram_tensor("y", [NT, C], fp32, kind="ExternalOutput")

    with tile.TileContext(nc) as tc, \
         tc.tile_pool(name="consts", bufs=1) as consts, \
         tc.tile_pool(name="stage", bufs=3) as stage, \
         tc.tile_pool(name="persist", bufs=1) as persist, \
         tc.tile_pool(name="biasp", bufs=4) as biasp, \
         tc.tile_pool(name="soft", bufs=3) as soft, \
         tc.tile_pool(name="small", bufs=8) as small, \
         tc.tile_pool(name="ypool", bufs=3) as ypool, \
         tc.tile_pool(name="psA", bufs=4, space="PSUM") as psA, \
         tc.tile_pool(name="psT", bufs=2, space="PSUM") as psT, \
         tc.tile_pool(name="psO", bufs=2, space="PSUM") as psO:

        # ----- constants -----
        bq_sb = consts.tile([P, KO], fp32, name="bq_sb")
        nc.sync.dma_start(out=bq_sb, in_=bqd.rearrange("(o p) -> p o", p=P))
        bk_sb = consts.tile([P, KO], fp32, name="bk_sb")
        nc.sync.dma_start(out=bk_sb, in_=bkd.rearrange("(o p) -> p o", p=P))
        bv_sb = consts.tile([P, C], fp32, name="bv_sb")
        bv_ap = bvd[:]
        nc.sync.dma_start(
            out=bv_sb,
            in_=bass.AP(tensor=bv_ap.tensor, offset=bv_ap.offset, ap=[[0, P], [1, C]]),
        )
        bo_sb = consts.tile([P, C], fp32, name="bo_sb")
        bo_ap = bod[:]
        nc.sync.dma_start(
            out=bo_sb,
            in_=bass.AP(tensor=bo_ap.tensor, offset=bo_ap.offset, ap=[[0, P], [1, C]]),
        )
        ident = consts.tile([P, P], fp32, name="ident")
        make_identity(nc, ident)

        # ----- load inputs, cast to bf16 -----
        def load_cast(dram, name):
            t_bf = persist.tile([P, KO, C], bf16, name=name)
            for ko in range(KO):
                st = stage.tile([P, C], fp32, tag="stage")
                nc.sync.dma_start(out=st, in_=dram[ko * P : (ko + 1) * P, :])
                nc.any.tensor_copy(out=t_bf[:, ko, :], in_=st)
            return t_bf

        xT_bf = load_cast(xT, "xT_bf")
        wq_bf = load_cast(wqT, "wq_bf")
        wk_bf = load_cast(wkT, "wk_bf")
        wv_bf = load_cast(wvT, "wv_bf")
        wo_bf = load_cast(woT, "wo_bf")

        qT_bf = persist.tile([P, KO, NT], bf16, name="qT_bf")
        kT_bf = persist.tile([P, KO, NT], bf16, name="kT_bf")
        vv_bf = persist.tile([P, NT // P, C], bf16, name="vv_bf")
        oT_bf = persist.tile([P, KO, NT], bf16, name="oT_bf")

        NCH = NT // 512  # 2 free-dim chunks of 512

        # ----- Q/K projections -> transposed layout [c_out on partitions, t]
        # q is pre-scaled by 1/sqrt(D) (bq comes pre-scaled from host).
        for w_bf, out_bf, b_sb, scl in (
            (wq_bf, qT_bf, bq_sb, 1.0 / np.sqrt(D)),
            (wk_bf, kT_bf, bk_sb, 1.0),
        ):
            for mo in range(KO):
                for nch in range(NCH):
                    ps = psA.tile([P, 512], fp32, tag="psA")
                    for ko in range(KO):
                        nc.tensor.matmul(
                            ps,
                            lhsT=w_bf[:, ko, mo * P : (mo + 1) * P],
                            rhs=xT_bf[:, ko, nch * 512 : (nch + 1) * 512],
                            start=(ko == 0),
                            stop=(ko == KO - 1),
                        )
                    nc.scalar.activation(
                        out=out_bf[:, mo, nch * 512 : (nch + 1) * 512],
                        in_=ps,
                        func=Act.Identity,
                        bias=b_sb[:, mo : mo + 1],
                        scale=scl,
                    )

        # ----- V projection -> natural layout [t on partitions, c_out]
        for to in range(NT // P):
            for nch in range(NCH):
                ps = psA.tile([P, 512], fp32, tag="psA")
                for ko in range(KO):
                    nc.tensor.matmul(
                        ps,
                        lhsT=xT_bf[:, ko, to * P : (to + 1) * P],
                        rhs=wv_bf[:, ko, nch * 512 : (nch + 1) * 512],
                        start=(ko == 0),
                        stop=(ko == KO - 1),
                    )
                nc.any.tensor_add(
                    out=vv_bf[:, to, nch * 512 : (nch + 1) * 512],
                    in0=ps,
                    in1=bv_sb[:, nch * 512 : (nch + 1) * 512],
                )

        # ----- attention -----
        for b in range(B_LOC):
            for h in range(H):
                po = (h % 2) * D  # partition offset of this head's dims
                mo = h // 2  # which 128-block of c holds this head pair
                qh = qT_bf[po : po + D, mo, b * T : (b + 1) * T]  # [64, 512]
                kh = kT_bf[po : po + D, mo, b * T : (b + 1) * T]  # [64, 512]
                for i in range(TB):
                    w = P * (i + 1)  # causal key width for this q block
                    psS = psA.tile([P, 512], fp32, tag="psA")
                    nc.tensor.matmul(
                        psS[:, :w],
                        lhsT=qh[:, i * P : (i + 1) * P],
                        rhs=kh[:, :w],
                        start=True,
                        stop=True,
                    )
                    bias_sb = biasp.tile([P, 512], fp32, tag="bias")
                    nc.sync.dma_start(
                        out=bias_sb[:, :w], in_=biasm[h, i * P : (i + 1) * P, :w]
                    )
                    nc.any.tensor_add(
                        out=psS[:, :w], in0=psS[:, :w], in1=bias_sb[:, :w]
                    )
                    # softmax (unnormalized exp + row sum; values bounded,
                    # max-subtraction unnecessary: |S| <~ 10, exp safe)
                    Pf = soft.tile([P, 512], fp32, tag="Pf")
                    lr = small.tile([P, 2], fp32, tag="lr")
                    nc.scalar.activation(
                        out=Pf[:, :w],
                        in_=psS[:, :w],
                        func=Act.Exp,
                        accum_out=lr[:, 0:1],
                    )
                    nc.vector.reciprocal(out=lr[:, 1:2], in_=lr[:, 0:1])
                    nc.any.tensor_scalar_mul(
                        out=Pf[:, :w], in0=Pf[:, :w], scalar1=lr[:, 1:2]
                    )
                    # transpose P 128x128 blocks on TensorE -> bf16
                    PT = soft.tile([P, 512], bf16, tag="PT")
                    for j in range(i + 1):
                        pst = psT.tile([P, P], fp32, tag="psT")
                        nc.tensor.transpose(
                            pst, Pf[:, j * P : (j + 1) * P], ident
                        )
                        nc.any.tensor_copy(
                            out=PT[:, j * P : (j + 1) * P], in_=pst
                        )
                    # AV: o^T[d, t] accumulated over key blocks
                    psv = psO.tile([D, P], fp32, tag="psO")
                    for j in range(i + 1):
                        nc.tensor.matmul(
                            psv,
                            lhsT=vv_bf[:, b * TB + j, h * D : (h + 1) * D],
                            rhs=PT[:, j * P : (j + 1) * P],
                            start=(j == 0),
                            stop=(j == i),
                        )
                    nc.any.tensor_copy(
                        out=oT_bf[po : po + D, mo, b * T + i * P : b * T + (i + 1) * P],
                        in_=psv,
                    )

        # ----- output projection -> y [t, c_out] fp32
        for to in range(NT // P):
            for nch in range(NCH):
                ps = psA.tile([P, 512], fp32, tag="psA")
                for co in range(KO):
                    nc.tensor.matmul(
                        ps,
                        lhsT=oT_bf[:, co, to * P : (to + 1) * P],
                        rhs=wo_bf[:, co, nch * 512 : (nch + 1) * 512],
                        start=(co == 0),
                        stop=(co == KO - 1),
                    )
                ysb = ypool.tile([P, 512], fp32, tag="y")
                nc.any.tensor_add(
                    out=ysb, in0=ps, in1=bo_sb[:, nch * 512 : (nch + 1) * 512]
                )
                nc.sync.dma_start(
                    out=y[to * P : (to + 1) * P, nch * 512 : (nch + 1) * 512],
                    in_=ysb,
                )

    if split_waits:
        _split_big_waits(nc, mybir, limit=1)
    return nc


def make_in_maps(inputs):
    x = np.ascontiguousarray(np.asarray(inputs["x"], dtype=np.float32))
    wT = {
        k: np.ascontiguousarray(np.asarray(inputs[f"W{k}"], dtype=np.float32).T)
        for k in "qkvo"
    }
    bq = np.asarray(inputs["bq"], dtype=np.float32) * np.float32(1.0 / np.sqrt(D))
    bk = np.asarray(inputs["bk"], dtype=np.float32)
    bv = np.asarray(inputs["bv"], dtype=np.float32)
    bo = np.asarray(inputs["bo"], dtype=np.float32)
    bm = np.asarray(inputs["rel_pos_bias"], dtype=np.float32)[:, :T, :T].copy()
    iu = np.triu_indices(T, 1)
    bm[:, iu[0], iu[1]] = NEG
    bm = np.ascontiguousarray(bm)

    xT_all = x.reshape(N_CORES, NT, C).transpose(0, 2, 1)
    in_maps = []
    for c in range(N_CORES):
        in_maps.append(
            {
                "xT": np.ascontiguousarray(xT_all[c]),
                "wqT": wT["q"],
                "wkT": wT["k"],
                "wvT": wT["v"],
                "woT": wT["o"],
                "bq": bq,
                "bk": bk,
                "bv": bv,
                "bo": bo,
                "biasm": bm,
            }
        )
    return in_maps


def get_runner():
    """Build the program once and return a callable in_maps -> per-core
    outputs, holding onto the compiled jax executable across calls."""
    if "runner" in _CACHE:
        return _CACHE["runner"]
    from concourse import bass2jax

    nc = build_program()

    def runner(in_maps):
        return bass2jax.run_bass_via_pjrt(nc, in_maps, n_cores=N_CORES)

    _CACHE["runner"] = runner
    _CACHE["nc"] = nc
    return runner


def kernel(**inputs) -> np.ndarray:
    runner = get_runner()
    in_maps = make_in_maps(inputs)
    results = runner(in_maps)
    out = np.concatenate(
        [results[c]["y"].reshape(B_LOC, T, C) for c in range(N_CORES)], axis=0
    )
    return out.astype(np.float32)


# revision 24
# speedup vs baseline: 4.0446x; 4.0446x over previous
"""Trainium2 Bass kernel for nn_MultiHeadAttention_377957122345.

B=16, T=512, C=1024, H=16, D=64.  Data-parallel over batch: each of the
8 NeuronCores computes attention for 2 sequences; no collectives.

Per-core device program (SPMD, identical on all cores):
  - inputs staged on host as transposed layouts: xT [C, 1024] (c_in on
    partitions), W^T [c_in, c_out] for all four projections, and
    rel_pos_bias with the causal mask folded in (-1e30 above diagonal).
  - all matmuls in bf16 with fp32 PSUM accumulation; softmax in fp32.
  - Q/K projections produce q^T/k^T (head_dim on partitions); V and the
    final output are produced in natural [t, c] layout.
  - causal block skipping: for query block i only key blocks j <= i are
    computed, masked entries never touched.
  - scores S = q^T.T @ k^T land in PSUM [128 t, w s]; bias added in
    PSUM; exp on ScalarE with row-sum accumulation; P normalized and
    transposed 128x128 via TensorE so the AV matmul can contract over s.
"""

import numpy as np

B, T, C, H = 16, 512, 1024, 16
D = C // H  # 64
N_CORES = 8
B_LOC = B // N_CORES  # 2 sequences per core
NT = B_LOC * T  # 1024 tokens per core
P = 128
KO = C // P  # 8 contraction subtiles
TB = T // P  # 4 query blocks per sequence
NEG = -1e30

_CACHE = {}

import contextlib


def _nullcm():
    return contextlib.nullcontext()



def _split_big_waits(nc, mybir, limit=1):
    # This walrus build rejects instructions whose sync_info.on_wait
    # exceeds its slot count (the Tile end-of-kernel Drain trips it).
    # Move excess waits onto dedicated same-engine NOPs placed directly
    # before the instruction; the engine stalls on those first, so the
    # semantics are unchanged.
    f = nc.m.functions[0]
    for bb in f.blocks:
        out = []
        changed = False
        for inst in bb.instructions:
            si = getattr(inst, "sync_info", None)
            waits = list(si.on_wait) if si is not None and si.on_wait else []
            if len(waits) > limit:
                changed = True
                head, tail = waits[:-limit], waits[-limit:]
                for k in range(0, len(head), limit):
                    out.append(
                        mybir.InstNoOp(
                            name=f"I-{nc.next_id()}",
                            sync_info=mybir.SyncInfo(
                                on_wait=head[k : k + limit], on_update=[]
                            ),
                            bass_nofuse=True,
                            engine=inst.engine,
                        )
                    )
                si.on_wait = tail
            out.append(inst)
        if changed:
            bb.instructions = out
    return nc


def build_program(split_waits=True, reps=1, skip_attn=False, skip_proj=False, dbg=False, bias_dma_psum=False, attn_lvl=3):
    import concourse.bass as bass
    import concourse.mybir as mybir
    import concourse.tile as tile

    fp32 = mybir.dt.float32
    bf16 = mybir.dt.bfloat16
    Act = mybir.ActivationFunctionType

    nc = bass.Bass()
    xT = nc.dram_tensor("xT", [C, NT], fp32, kind="ExternalInput")
    wqT = nc.dram_tensor("wqT", [C, C], fp32, kind="ExternalInput")
    wkT = nc.dram_tensor("wkT", [C, C], fp32, kind="ExternalInput")
    wvT = nc.dram_tensor("wvT", [C, C], fp32, kind="ExternalInput")
    woT = nc.dram_tensor("woT", [C, C], fp32, kind="ExternalInput")
    bqd = nc.dram_tensor("bq", [C], fp32, kind="ExternalInput")
    bkd = nc.dram_tensor("bk", [C], fp32, kind="ExternalInput")
    bvd = nc.dram_tensor("bv", [C], fp32, kind="ExternalInput")
    bod = nc.dram_tensor("bo", [C], fp32, kind="ExternalInput")
    biasm = nc.dram_tensor("biasm", [H, T, T], bf16, kind="ExternalInput")
    y = nc.dram_tensor("y", [NT, C], fp32, kind="ExternalOutput")

    with tile.TileContext(nc) as tc, \
         tc.tile_pool(name="consts", bufs=1) as consts, \
         tc.tile_pool(name="stage", bufs=2) as stage, \
         tc.tile_pool(name="persist", bufs=1) as persist, \
         tc.tile_pool(name="biasp", bufs=4) as biasp, \
         tc.tile_pool(name="soft", bufs=3) as soft, \
         tc.tile_pool(name="small", bufs=8) as small, \
         tc.tile_pool(name="ypool", bufs=3) as ypool, \
         tc.tile_pool(name="psA", bufs=4, space="PSUM") as psA, \
         tc.tile_pool(name="psO", bufs=3, space="PSUM") as psO, \
         (tc.For_i(0, reps, 1) if reps > 1 else _nullcm()):

        # ----- constants -----
        bq_sb = consts.tile([P, KO], fp32, name="bq_sb")
        nc.sync.dma_start(out=bq_sb, in_=bqd.rearrange("(o p) -> p o", p=P))
        bk_sb = consts.tile([P, KO], fp32, name="bk_sb")
        nc.sync.dma_start(out=bk_sb, in_=bkd.rearrange("(o p) -> p o", p=P))
        bv_sb = consts.tile([P, C], fp32, name="bv_sb")
        bv_ap = bvd[:]
        nc.sync.dma_start(
            out=bv_sb,
            in_=bass.AP(tensor=bv_ap.tensor, offset=bv_ap.offset, ap=[[0, P], [1, C]]),
        )
        bo_sb = consts.tile([P, C], fp32, name="bo_sb")
        bo_ap = bod[:]
        nc.sync.dma_start(
            out=bo_sb,
            in_=bass.AP(tensor=bo_ap.tensor, offset=bo_ap.offset, ap=[[0, P], [1, C]]),
        )

        # ----- load inputs, cast to bf16 -----
        def load_cast(dram, name):
            t_bf = persist.tile([P, KO, C], bf16, name=name)
            for ko in range(KO):
                st = stage.tile([P, C], fp32, tag="stage")
                nc.sync.dma_start(out=st, in_=dram[ko * P : (ko + 1) * P, :])
                nc.any.tensor_copy(out=t_bf[:, ko, :], in_=st)
            return t_bf

        xT_bf = load_cast(xT, "xT_bf")
        wq_bf = load_cast(wqT, "wq_bf")
        wk_bf = load_cast(wkT, "wk_bf")
        wv_bf = load_cast(wvT, "wv_bf")
        wo_bf = load_cast(woT, "wo_bf")

        qT_bf = persist.tile([P, KO, NT], bf16, name="qT_bf")
        kT_bf = persist.tile([P, KO, NT], bf16, name="kT_bf")
        vaug = persist.tile([P, NT // P, H * (D + 1)], bf16, name="vaug")
        oT_bf = persist.tile([P, KO, NT], bf16, name="oT_bf")

        NCH = NT // 512  # 2 free-dim chunks of 512

        # ----- Q/K projections -> transposed layout [c_out on partitions, t]
        # q is pre-scaled by 1/sqrt(D) (bq comes pre-scaled from host).
        if skip_proj:
            nc.any.memset(qT_bf[:], 0.0)
            nc.any.memset(kT_bf[:], 0.0)
            nc.any.memset(vv_bf[:], 0.0)
        for w_bf, out_bf, b_sb, scl in (() if skip_proj else (
            (wq_bf, qT_bf, bq_sb, 1.0 / np.sqrt(D)),
            (wk_bf, kT_bf, bk_sb, 1.0),
        )):
            for mo in range(KO):
                for nch in range(NCH):
                    ps = psA.tile([P, 512], fp32, tag="psA")
                    for ko in range(KO):
                        nc.tensor.matmul(
                            ps,
                            lhsT=w_bf[:, ko, mo * P : (mo + 1) * P],
                            rhs=xT_bf[:, ko, nch * 512 : (nch + 1) * 512],
                            start=(ko == 0),
                            stop=(ko == KO - 1),
                        )
                    nc.scalar.activation(
                        out=out_bf[:, mo, nch * 512 : (nch + 1) * 512],
                        in_=ps,
                        func=Act.Identity,
                        bias=b_sb[:, mo : mo + 1],
                        scale=scl,
                    )

        # ----- V projection -> ones-augmented layout: head h occupies
        # columns [h*65, h*65+64) with a ones column at h*65+64, so the AV
        # matmul emits the softmax denominator as PSUM row 64.
        nc.any.memset(vaug[:], 1.0)
        for to in range(NT // P if not skip_proj else 0):
            for nch in range(NCH):
                ps = psA.tile([P, 512], fp32, tag="psA")
                for ko in range(KO):
                    nc.tensor.matmul(
                        ps,
                        lhsT=xT_bf[:, ko, to * P : (to + 1) * P],
                        rhs=wv_bf[:, ko, nch * 512 : (nch + 1) * 512],
                        start=(ko == 0),
                        stop=(ko == KO - 1),
                    )
                for hh in range(8):
                    h = nch * 8 + hh
                    nc.any.tensor_add(
                        out=vaug[:, to, h * (D + 1) : h * (D + 1) + D],
                        in0=ps[:, hh * D : (hh + 1) * D],
                        in1=bv_sb[:, h * D : (h + 1) * D],
                    )

        r_dram = nc.dram_tensor("r_scratch", [H, NT], fp32)
        rd = r_dram[:]

        # ----- attention (S computed transposed: [s on partitions, t]) --
        if skip_attn or attn_lvl < 3:
            nc.any.memset(oT_bf[:], 0.0)
        for b in range(B_LOC if not skip_attn else 0):
            for h in range(H):
                po = (h % 2) * D  # partition offset of this head's dims
                mo = h // 2  # which 128-block of c holds this head pair
                qh = qT_bf[po : po + D, mo, b * T : (b + 1) * T]  # [64, 512]
                kh = kT_bf[po : po + D, mo, b * T : (b + 1) * T]  # [64, 512]
                # S^T_j = k_j @ q^T for t >= j*128 (causal), +bias, exp.
                # Unnormalized probabilities; denominator comes from the
                # ones column in vaug during AV.
                PTs = []
                for j in range(TB):
                    wj = T - j * P
                    psS = psA.tile([P, 512], fp32, tag="psA")
                    nc.tensor.matmul(
                        psS[:, :wj],
                        lhsT=kh[:, j * P : (j + 1) * P],
                        rhs=qh[:, j * P :],
                        start=True,
                        stop=True,
                    )
                    expb_sb = biasp.tile([P, 512], bf16, tag="bias", bufs=5)
                    nc.sync.dma_start(
                        out=expb_sb[:, :wj],
                        in_=biasm[h, j * P : (j + 1) * P, j * P :],
                    )
                    PT0 = soft.tile([P, 512], bf16, tag="PT0", bufs=6)
                    nc.scalar.activation(
                        out=PT0[:, :wj], in_=psS[:, :wj], func=Act.Exp
                    )
                    PT = soft.tile([P, 512], bf16, tag="PT", bufs=8)
                    nc.any.tensor_mul(
                        out=PT[:, :wj], in0=PT0[:, :wj], in1=expb_sb[:, :wj]
                    )
                    PTs.append(PT)
                if attn_lvl < 2:
                    continue
                r_sb = small.tile([1, T], fp32, tag="r", bufs=3)
                for i in range(TB):
                    pst = psO.tile([D + 1, P], fp32, tag="psO")
                    for j in range(i + 1):
                        nc.tensor.matmul(
                            pst,
                            lhsT=vaug[:, b * TB + j, h * (D + 1) : (h + 1) * (D + 1)],
                            rhs=PTs[j][:, (i - j) * P : (i - j + 1) * P],
                            start=(j == 0),
                            stop=(j == i),
                        )
                    nc.vector.reciprocal(
                        out=r_sb[0:1, i * P : (i + 1) * P],
                        in_=pst[D : D + 1, :],
                    )
                    nc.any.tensor_copy(
                        out=oT_bf[po : po + D, mo, b * T + i * P : b * T + (i + 1) * P],
                        in_=pst[:D, :],
                    )
                nc.sync.dma_start(
                    out=r_dram[h, b * T : (b + 1) * T], in_=r_sb[0:1, :]
                )

        # ---- batched softmax normalization: broadcast the packed 1/l
        # DRAM scratch into the oT layout with two replicating DMAs (DRAM
        # sources allow step-0 dims), then one in-place multiply.
        if not skip_attn and attn_lvl >= 3:
            r_bc = soft.tile([P, KO, NT // 2], fp32, name="r_bc", bufs=1)
            for half in range(2):
                for tch in range(2):
                    nc.sync.dma_start(
                        out=r_bc[half * D : (half + 1) * D],
                        in_=bass.AP(
                            tensor=rd.tensor,
                            offset=rd.offset + half * NT + tch * (NT // 2),
                            ap=[[0, D], [2 * NT, KO], [1, NT // 2]],
                        ),
                    )
                    nc.any.tensor_mul(
                        out=oT_bf[
                            half * D : (half + 1) * D,
                            :,
                            tch * (NT // 2) : (tch + 1) * (NT // 2),
                        ],
                        in0=oT_bf[
                            half * D : (half + 1) * D,
                            :,
                            tch * (NT // 2) : (tch + 1) * (NT // 2),
                        ],
                        in1=r_bc[half * D : (half + 1) * D],
                    )

        if dbg:
            for nm, tl in (("qT_dbg", qT_bf), ("kT_dbg", kT_bf),
                           ("vv_dbg", vv_bf), ("oT_dbg", oT_bf)):
                dt_ = nc.dram_tensor(nm, list(tl.shape), bf16, kind="ExternalOutput")
                nc.sync.dma_start(out=dt_[:], in_=tl[:])

        # ----- output projection -> y [t, c_out] fp32
        for to in range(NT // P):
            for nch in range(NCH):
                ps = psA.tile([P, 512], fp32, tag="psA")
                for co in range(KO):
                    nc.tensor.matmul(
                        ps,
                        lhsT=oT_bf[:, co, to * P : (to + 1) * P],
                        rhs=wo_bf[:, co, nch * 512 : (nch + 1) * 512],
                        start=(co == 0),
                        stop=(co == KO - 1),
                    )
                ysb = ypool.tile([P, 512], fp32, tag="y")
                nc.any.tensor_add(
                    out=ysb, in0=ps, in1=bo_sb[:, nch * 512 : (nch + 1) * 512]
                )
                nc.sync.dma_start(
                    out=y[to * P : (to + 1) * P, nch * 512 : (nch + 1) * 512],
                    in_=ysb,
                )

    if split_waits:
        _split_big_waits(nc, mybir, limit=1)
    return nc


def make_in_maps(inputs):
    x = np.ascontiguousarray(np.asarray(inputs["x"], dtype=np.float32))
    wT = {
        k: np.ascontiguousarray(np.asarray(inputs[f"W{k}"], dtype=np.float32).T)
        for k in "qkvo"
    }
    bq = np.asarray(inputs["bq"], dtype=np.float32) * np.float32(1.0 / np.sqrt(D))
    bk = np.asarray(inputs["bk"], dtype=np.float32)
    bv = np.asarray(inputs["bv"], dtype=np.float32)
    bo = np.asarray(inputs["bo"], dtype=np.float32)
    import ml_dtypes

    bm = np.asarray(inputs["rel_pos_bias"], dtype=np.float32)[:, :T, :T].copy()
    iu = np.triu_indices(T, 1)
    bm[:, iu[0], iu[1]] = NEG
    # multiplicative form: exp(S+bias) = exp(S) * exp(bias); causal mask
    # becomes an exact multiplicative zero. Transposed to [h, s, t].
    bm = np.ascontiguousarray(
        np.exp(bm.transpose(0, 2, 1)).astype(ml_dtypes.bfloat16)
    )

    xT_all = x.reshape(N_CORES, NT, C).transpose(0, 2, 1)
    in_maps = []
    for c in range(N_CORES):
        in_maps.append(
            {
                "xT": np.ascontiguousarray(xT_all[c]),
                "wqT": wT["q"],
                "wkT": wT["k"],
                "wvT": wT["v"],
                "woT": wT["o"],
                "bq": bq,
                "bk": bk,
                "bv": bv,
                "bo": bo,
                "biasm": bm,
            }
        )
    return in_maps


def build_jitted(nc, n_cores=N_CORES):
    """Build a persistent jitted shard_map executable for `nc` (the
    multi-core path of bass2jax.run_bass_via_pjrt, kept resident so repeat
    kernel() calls skip retracing)."""
    import jax
    from jax.experimental.shard_map import shard_map
    from jax.sharding import Mesh, NamedSharding, PartitionSpec

    from concourse import mybir
    from concourse.bass2jax import (
        _bass_exec_p,
        install_neuronx_cc_hook,
        partition_id_tensor,
    )

    install_neuronx_cc_hook()
    partition_name = nc.partition_id_tensor.name if nc.partition_id_tensor else None

    in_names, out_names, out_avals, zero_outs = [], [], [], []
    for alloc in nc.m.functions[0].allocations:
        if not isinstance(alloc, mybir.MemoryLocationSet):
            continue
        name = alloc.memorylocations[0].name
        if alloc.kind == "ExternalInput":
            if name != partition_name:
                in_names.append(name)
        elif alloc.kind == "ExternalOutput":
            out_names.append(name)
            shape = tuple(alloc.tensor_shape)
            dtype = mybir.dt.np(alloc.dtype)
            out_avals.append(jax.core.ShapedArray(shape, dtype))
            zero_outs.append(np.zeros(shape, dtype))
    n_params = len(in_names)
    n_outs = len(out_avals)
    all_in_names = list(in_names) + list(out_names)
    if partition_name is not None:
        all_in_names.append(partition_name)
    donate = tuple(range(n_params, n_params + n_outs))

    def _body(*args):
        operands = list(args)
        if partition_name is not None:
            operands.append(partition_id_tensor())
        outs = _bass_exec_p.bind(
            *operands,
            out_avals=tuple(out_avals),
            in_names=tuple(all_in_names),
            out_names=tuple(out_names),
            lowering_input_output_aliases=(),
            sim_require_finite=True,
            sim_require_nnan=True,
            nc=nc,
        )
        return tuple(outs)

    devices = jax.devices()[:n_cores]
    mesh = Mesh(np.asarray(devices), ("core",))
    in_specs = (PartitionSpec("core"),) * (n_params + n_outs)
    out_specs = (PartitionSpec("core"),) * n_outs
    jitted = jax.jit(
        shard_map(_body, mesh=mesh, in_specs=in_specs, out_specs=out_specs,
                  check_rep=False),
        donate_argnums=donate,
        keep_unused=True,
    )
    sharding = NamedSharding(mesh, PartitionSpec("core"))
    return jitted, in_names, out_names, out_avals, zero_outs, sharding


def get_runner():
    """Build the program + executable once; return in_maps -> per-core
    output dicts."""
    if "runner" in _CACHE:
        return _CACHE["runner"]
    import jax

    nc = build_program()
    jitted, in_names, out_names, out_avals, zero_outs, sharding = build_jitted(nc)
    n_cores = N_CORES

    def runner(in_maps):
        concat_in = [
            jax.device_put(
                np.concatenate(
                    [np.asarray(in_maps[c][nm]) for c in range(n_cores)], axis=0
                ),
                sharding,
            )
            for nm in in_names
        ]
        zeros = [
            jax.device_put(
                np.zeros((n_cores * z.shape[0], *z.shape[1:]), z.dtype), sharding
            )
            for z in zero_outs
        ]
        out_arrs = jitted(*concat_in, *zeros)
        return [
            {
                nm: np.asarray(out_arrs[i]).reshape(n_cores, *out_avals[i].shape)[c]
                for i, nm in enumerate(out_names)
            }
            for c in range(n_cores)
        ]

    _CACHE["runner"] = runner
    _CACHE["nc"] = nc
    return runner


def kernel(**inputs) -> np.ndarray:
    runner = get_runner()
    in_maps = make_in_maps(inputs)
    results = runner(in_maps)
    out = np.concatenate(
        [results[c]["y"].reshape(B_LOC, T, C) for c in range(N_CORES)], axis=0
    )
    return out.astype(np.float32)


# revision 25
# speedup vs baseline: 4.7928x; 1.1850x over previous
"""Trainium2 Bass kernel for nn_MultiHeadAttention_377957122345.

B=16, T=512, C=1024, H=16, D=64.  Data-parallel over batch: each of the
8 NeuronCores computes attention for 2 sequences; no collectives.

Per-core device program (SPMD, identical on all cores):
  - inputs staged on host as transposed layouts: xT [C, 1024] (c_in on
    partitions), W^T [c_in, c_out] for all four projections, and
    rel_pos_bias with the causal mask folded in (-1e30 above diagonal).
  - all matmuls in bf16 with fp32 PSUM accumulation; softmax in fp32.
  - Q/K projections produce q^T/k^T (head_dim on partitions); V and the
    final output are produced in natural [t, c] layout.
  - causal block skipping: for query block i only key blocks j <= i are
    computed, masked entries never touched.
  - scores S = q^T.T @ k^T land in PSUM [128 t, w s]; bias added in
    PSUM; exp on ScalarE with row-sum accumulation; P normalized and
    transposed 128x128 via TensorE so the AV matmul can contract over s.
"""

import numpy as np

B, T, C, H = 16, 512, 1024, 16
D = C // H  # 64
N_CORES = 8
B_LOC = B // N_CORES  # 2 sequences per core
NT = B_LOC * T  # 1024 tokens per core
P = 128
KO = C // P  # 8 contraction subtiles
TB = T // P  # 4 query blocks per sequence
NEG = -1e30

_CACHE = {}

import contextlib


def _nullcm():
    return contextlib.nullcontext()



def _split_big_waits(nc, mybir, limit=1):
    # This walrus build rejects instructions whose sync_info.on_wait
    # exceeds its slot count (the Tile end-of-kernel Drain trips it).
    # Move excess waits onto dedicated same-engine NOPs placed directly
    # before the instruction; the engine stalls on those first, so the
    # semantics are unchanged.
    f = nc.m.functions[0]
    for bb in f.blocks:
        out = []
        changed = False
        for inst in bb.instructions:
            si = getattr(inst, "sync_info", None)
            waits = list(si.on_wait) if si is not None and si.on_wait else []
            if len(waits) > limit:
                changed = True
                head, tail = waits[:-limit], waits[-limit:]
                for k in range(0, len(head), limit):
                    out.append(
                        mybir.InstNoOp(
                            name=f"I-{nc.next_id()}",
                            sync_info=mybir.SyncInfo(
                                on_wait=head[k : k + limit], on_update=[]
                            ),
                            bass_nofuse=True,
                            engine=inst.engine,
                        )
                    )
                si.on_wait = tail
            out.append(inst)
        if changed:
            bb.instructions = out
    return nc


def build_program(split_waits=True, reps=1, skip_attn=False, skip_proj=False, dbg=False, bias_dma_psum=False, attn_lvl=3):
    import concourse.bass as bass
    import concourse.mybir as mybir
    import concourse.tile as tile

    fp32 = mybir.dt.float32
    bf16 = mybir.dt.bfloat16
    Act = mybir.ActivationFunctionType

    nc = bass.Bass()
    xT = nc.dram_tensor("xT", [C, NT], fp32, kind="ExternalInput")
    wqT = nc.dram_tensor("wqT", [C, C], fp32, kind="ExternalInput")
    wkT = nc.dram_tensor("wkT", [C, C], fp32, kind="ExternalInput")
    wvT = nc.dram_tensor("wvT", [C, C], fp32, kind="ExternalInput")
    woT = nc.dram_tensor("woT", [C, C], fp32, kind="ExternalInput")
    bqd = nc.dram_tensor("bq", [C], fp32, kind="ExternalInput")
    bkd = nc.dram_tensor("bk", [C], fp32, kind="ExternalInput")
    bvd = nc.dram_tensor("bv", [C], fp32, kind="ExternalInput")
    bod = nc.dram_tensor("bo", [C], fp32, kind="ExternalInput")
    biasm = nc.dram_tensor("biasm", [H, T, T], bf16, kind="ExternalInput")
    y = nc.dram_tensor("y", [NT, C], fp32, kind="ExternalOutput")

    with tile.TileContext(nc) as tc, \
         tc.tile_pool(name="consts", bufs=1) as consts, \
         tc.tile_pool(name="stage", bufs=2) as stage, \
         tc.tile_pool(name="persist", bufs=1) as persist, \
         tc.tile_pool(name="biasp", bufs=4) as biasp, \
         tc.tile_pool(name="soft", bufs=3) as soft, \
         tc.tile_pool(name="small", bufs=8) as small, \
         tc.tile_pool(name="ypool", bufs=3) as ypool, \
         tc.tile_pool(name="psA", bufs=5, space="PSUM") as psA, \
         tc.tile_pool(name="psO", bufs=3, space="PSUM") as psO, \
         (tc.For_i(0, reps, 1) if reps > 1 else _nullcm()):

        # ----- constants -----
        bq_sb = consts.tile([P, KO], fp32, name="bq_sb")
        nc.sync.dma_start(out=bq_sb, in_=bqd.rearrange("(o p) -> p o", p=P))
        bk_sb = consts.tile([P, KO], fp32, name="bk_sb")
        nc.sync.dma_start(out=bk_sb, in_=bkd.rearrange("(o p) -> p o", p=P))
        bv_sb = consts.tile([P, C], fp32, name="bv_sb")
        bv_ap = bvd[:]
        nc.sync.dma_start(
            out=bv_sb,
            in_=bass.AP(tensor=bv_ap.tensor, offset=bv_ap.offset, ap=[[0, P], [1, C]]),
        )
        bo_sb = consts.tile([P, C], fp32, name="bo_sb")
        bo_ap = bod[:]
        nc.sync.dma_start(
            out=bo_sb,
            in_=bass.AP(tensor=bo_ap.tensor, offset=bo_ap.offset, ap=[[0, P], [1, C]]),
        )

        # ----- load inputs, cast to bf16 -----
        def load_cast(dram, name):
            t_bf = persist.tile([P, KO, C], bf16, name=name)
            for ko in range(KO):
                st = stage.tile([P, C], fp32, tag="stage")
                nc.sync.dma_start(out=st, in_=dram[ko * P : (ko + 1) * P, :])
                nc.any.tensor_copy(out=t_bf[:, ko, :], in_=st)
            return t_bf

        xT_bf = load_cast(xT, "xT_bf")
        wq_bf = load_cast(wqT, "wq_bf")
        wk_bf = load_cast(wkT, "wk_bf")
        wv_bf = load_cast(wvT, "wv_bf")
        wo_bf = load_cast(woT, "wo_bf")

        qT_bf = persist.tile([P, KO, NT], bf16, name="qT_bf")
        kT_bf = persist.tile([P, KO, NT], bf16, name="kT_bf")
        vaug = persist.tile([P, NT // P, H * (D + 1)], bf16, name="vaug")
        oT_bf = persist.tile([P, KO, NT], bf16, name="oT_bf")

        NCH = NT // 512  # 2 free-dim chunks of 512

        # ----- Q/K projections -> transposed layout [c_out on partitions, t]
        # q is pre-scaled by 1/sqrt(D) (bq comes pre-scaled from host).
        if skip_proj:
            nc.any.memset(qT_bf[:], 0.0)
            nc.any.memset(kT_bf[:], 0.0)
            nc.any.memset(vv_bf[:], 0.0)
        for w_bf, out_bf, b_sb, scl in (() if skip_proj else (
            (wq_bf, qT_bf, bq_sb, 1.0 / np.sqrt(D)),
            (wk_bf, kT_bf, bk_sb, 1.0),
        )):
            for mo in range(KO):
                for nch in range(NCH):
                    ps = psA.tile([P, 512], fp32, tag="psA")
                    for ko in range(KO):
                        nc.tensor.matmul(
                            ps,
                            lhsT=w_bf[:, ko, mo * P : (mo + 1) * P],
                            rhs=xT_bf[:, ko, nch * 512 : (nch + 1) * 512],
                            start=(ko == 0),
                            stop=(ko == KO - 1),
                        )
                    nc.scalar.activation(
                        out=out_bf[:, mo, nch * 512 : (nch + 1) * 512],
                        in_=ps,
                        func=Act.Identity,
                        bias=b_sb[:, mo : mo + 1],
                        scale=scl,
                    )

        # ----- V projection -> ones-augmented layout: head h occupies
        # columns [h*65, h*65+64) with a ones column at h*65+64, so the AV
        # matmul emits the softmax denominator as PSUM row 64.
        nc.any.memset(vaug[:], 1.0)
        for to in range(NT // P if not skip_proj else 0):
            for nch in range(NCH):
                ps = psA.tile([P, 512], fp32, tag="psA")
                for ko in range(KO):
                    nc.tensor.matmul(
                        ps,
                        lhsT=xT_bf[:, ko, to * P : (to + 1) * P],
                        rhs=wv_bf[:, ko, nch * 512 : (nch + 1) * 512],
                        start=(ko == 0),
                        stop=(ko == KO - 1),
                    )
                for hh in range(8):
                    h = nch * 8 + hh
                    nc.any.tensor_add(
                        out=vaug[:, to, h * (D + 1) : h * (D + 1) + D],
                        in0=ps[:, hh * D : (hh + 1) * D],
                        in1=bv_sb[:, h * D : (h + 1) * D],
                    )

        r_dram = nc.dram_tensor("r_scratch", [H, NT], fp32)
        rd = r_dram[:]

        # ----- attention (S computed transposed: [s on partitions, t]) --
        if skip_attn or attn_lvl < 3:
            nc.any.memset(oT_bf[:], 0.0)
        for b in range(B_LOC if not skip_attn else 0):
            for h in range(H):
                po = (h % 2) * D  # partition offset of this head's dims
                mo = h // 2  # which 128-block of c holds this head pair
                qh = qT_bf[po : po + D, mo, b * T : (b + 1) * T]  # [64, 512]
                kh = kT_bf[po : po + D, mo, b * T : (b + 1) * T]  # [64, 512]
                # S^T_j = k_j @ q^T for t >= j*128 (causal), +bias, exp.
                # Unnormalized probabilities; denominator comes from the
                # ones column in vaug during AV.
                PTs = []
                for j in range(TB):
                    wj = T - j * P
                    psS = psA.tile([P, 512], fp32, tag="psA")
                    nc.tensor.matmul(
                        psS[:, :wj],
                        lhsT=kh[:, j * P : (j + 1) * P],
                        rhs=qh[:, j * P :],
                        start=True,
                        stop=True,
                    )
                    expb_sb = biasp.tile([P, 512], bf16, tag="bias", bufs=5)
                    nc.sync.dma_start(
                        out=expb_sb[:, :wj],
                        in_=biasm[h, j * P : (j + 1) * P, j * P :],
                    )
                    PT0 = soft.tile([P, 512], bf16, tag="PT0", bufs=6)
                    nc.scalar.activation(
                        out=PT0[:, :wj], in_=psS[:, :wj], func=Act.Exp
                    )
                    PT = soft.tile([P, 512], bf16, tag="PT", bufs=8)
                    nc.any.tensor_mul(
                        out=PT[:, :wj], in0=PT0[:, :wj], in1=expb_sb[:, :wj]
                    )
                    PTs.append(PT)
                if attn_lvl < 2:
                    continue
                r_sb = small.tile([1, T], fp32, tag="r", bufs=3)
                for i in range(TB):
                    pst = psO.tile([D + 1, P], fp32, tag="psO")
                    for j in range(i + 1):
                        nc.tensor.matmul(
                            pst,
                            lhsT=vaug[:, b * TB + j, h * (D + 1) : (h + 1) * (D + 1)],
                            rhs=PTs[j][:, (i - j) * P : (i - j + 1) * P],
                            start=(j == 0),
                            stop=(j == i),
                        )
                    nc.vector.reciprocal(
                        out=r_sb[0:1, i * P : (i + 1) * P],
                        in_=pst[D : D + 1, :],
                    )
                    nc.any.tensor_copy(
                        out=oT_bf[po : po + D, mo, b * T + i * P : b * T + (i + 1) * P],
                        in_=pst[:D, :],
                    )
                nc.sync.dma_start(
                    out=r_dram[h, b * T : (b + 1) * T], in_=r_sb[0:1, :]
                )

        # ---- batched softmax normalization: broadcast the packed 1/l
        # DRAM scratch into the oT layout with two replicating DMAs (DRAM
        # sources allow step-0 dims), then one in-place multiply.
        if not skip_attn and attn_lvl >= 3:
            r_bc = soft.tile([P, KO, NT // 2], fp32, name="r_bc", bufs=1)
            for half in range(2):
                for tch in range(2):
                    nc.sync.dma_start(
                        out=r_bc[half * D : (half + 1) * D],
                        in_=bass.AP(
                            tensor=rd.tensor,
                            offset=rd.offset + half * NT + tch * (NT // 2),
                            ap=[[0, D], [2 * NT, KO], [1, NT // 2]],
                        ),
                    )
                    nc.any.tensor_mul(
                        out=oT_bf[
                            half * D : (half + 1) * D,
                            :,
                            tch * (NT // 2) : (tch + 1) * (NT // 2),
                        ],
                        in0=oT_bf[
                            half * D : (half + 1) * D,
                            :,
                            tch * (NT // 2) : (tch + 1) * (NT // 2),
                        ],
                        in1=r_bc[half * D : (half + 1) * D],
                    )

        if dbg:
            for nm, tl in (("qT_dbg", qT_bf), ("kT_dbg", kT_bf),
                           ("vv_dbg", vv_bf), ("oT_dbg", oT_bf)):
                dt_ = nc.dram_tensor(nm, list(tl.shape), bf16, kind="ExternalOutput")
                nc.sync.dma_start(out=dt_[:], in_=tl[:])

        # ----- output projection -> y [t, c_out] fp32
        for to in range(NT // P):
            for nch in range(NCH):
                ps = psA.tile([P, 512], fp32, tag="psA")
                for co in range(KO):
                    nc.tensor.matmul(
                        ps,
                        lhsT=oT_bf[:, co, to * P : (to + 1) * P],
                        rhs=wo_bf[:, co, nch * 512 : (nch + 1) * 512],
                        start=(co == 0),
                        stop=(co == KO - 1),
                    )
                ysb = ypool.tile([P, 512], fp32, tag="y")
                nc.any.tensor_add(
                    out=ysb, in0=ps, in1=bo_sb[:, nch * 512 : (nch + 1) * 512]
                )
                nc.sync.dma_start(
                    out=y[to * P : (to + 1) * P, nch * 512 : (nch + 1) * 512],
                    in_=ysb,
                )

    if split_waits:
        _split_big_waits(nc, mybir, limit=1)
    return nc


def make_in_maps(inputs):
    x = np.ascontiguousarray(np.asarray(inputs["x"], dtype=np.float32))
    wT = {
        k: np.ascontiguousarray(np.asarray(inputs[f"W{k}"], dtype=np.float32).T)
        for k in "qkvo"
    }
    bq = np.asarray(inputs["bq"], dtype=np.float32) * np.float32(1.0 / np.sqrt(D))
    bk = np.asarray(inputs["bk"], dtype=np.float32)
    bv = np.asarray(inputs["bv"], dtype=np.float32)
    bo = np.asarray(inputs["bo"], dtype=np.float32)
    import ml_dtypes

    bm = np.asarray(inputs["rel_pos_bias"], dtype=np.float32)[:, :T, :T].copy()
    iu = np.triu_indices(T, 1)
    bm[:, iu[0], iu[1]] = NEG
    # multiplicative form: exp(S+bias) = exp(S) * exp(bias); causal mask
    # becomes an exact multiplicative zero. Transposed to [h, s, t].
    bm = np.ascontiguousarray(
        np.exp(bm.transpose(0, 2, 1)).astype(ml_dtypes.bfloat16)
    )

    xT_all = x.reshape(N_CORES, NT, C).transpose(0, 2, 1)
    in_maps = []
    for c in range(N_CORES):
        in_maps.append(
            {
                "xT": np.ascontiguousarray(xT_all[c]),
                "wqT": wT["q"],
                "wkT": wT["k"],
                "wvT": wT["v"],
                "woT": wT["o"],
                "bq": bq,
                "bk": bk,
                "bv": bv,
                "bo": bo,
                "biasm": bm,
            }
        )
    return in_maps


def build_jitted(nc, n_cores=N_CORES):
    """Build a persistent jitted shard_map executable for `nc` (the
    multi-core path of bass2jax.run_bass_via_pjrt, kept resident so repeat
    kernel() calls skip retracing)."""
    import jax
    from jax.experimental.shard_map import shard_map
    from jax.sharding import Mesh, NamedSharding, PartitionSpec

    from concourse import mybir
    from concourse.bass2jax import (
        _bass_exec_p,
        install_neuronx_cc_hook,
        partition_id_tensor,
    )

    install_neuronx_cc_hook()
    partition_name = nc.partition_id_tensor.name if nc.partition_id_tensor else None

    in_names, out_names, out_avals, zero_outs = [], [], [], []
    for alloc in nc.m.functions[0].allocations:
        if not isinstance(alloc, mybir.MemoryLocationSet):
            continue
        name = alloc.memorylocations[0].name
        if alloc.kind == "ExternalInput":
            if name != partition_name:
                in_names.append(name)
        elif alloc.kind == "ExternalOutput":
            out_names.append(name)
            shape = tuple(alloc.tensor_shape)
            dtype = mybir.dt.np(alloc.dtype)
            out_avals.append(jax.core.ShapedArray(shape, dtype))
            zero_outs.append(np.zeros(shape, dtype))
    n_params = len(in_names)
    n_outs = len(out_avals)
    all_in_names = list(in_names) + list(out_names)
    if partition_name is not None:
        all_in_names.append(partition_name)
    donate = tuple(range(n_params, n_params + n_outs))

    def _body(*args):
        operands = list(args)
        if partition_name is not None:
            operands.append(partition_id_tensor())
        outs = _bass_exec_p.bind(
            *operands,
            out_avals=tuple(out_avals),
            in_names=tuple(all_in_names),
            out_names=tuple(out_names),
            lowering_input_output_aliases=(),
            sim_require_finite=True,
            sim_require_nnan=True,
            nc=nc,
        )
        return tuple(outs)

    devices = jax.devices()[:n_cores]
    mesh = Mesh(np.asarray(devices), ("core",))
    in_specs = (PartitionSpec("core"),) * (n_params + n_outs)
    out_specs = (PartitionSpec("core"),) * n_outs
    jitted = jax.jit(
        shard_map(_body, mesh=mesh, in_specs=in_specs, out_specs=out_specs,
                  check_rep=False),
        donate_argnums=donate,
        keep_unused=True,
    )
    sharding = NamedSharding(mesh, PartitionSpec("core"))
    return jitted, in_names, out_names, out_avals, zero_outs, sharding


def get_runner():
    """Build the program + executable once; return in_maps -> per-core
    output dicts."""
    if "runner" in _CACHE:
        return _CACHE["runner"]
    import jax

    nc = build_program()
    jitted, in_names, out_names, out_avals, zero_outs, sharding = build_jitted(nc)
    n_cores = N_CORES

    def runner(in_maps):
        concat_in = [
            jax.device_put(
                np.concatenate(
                    [np.asarray(in_maps[c][nm]) for c in range(n_cores)], axis=0
                ),
                sharding,
            )
            for nm in in_names
        ]
        zeros = [
            jax.device_put(
                np.zeros((n_cores * z.shape[0], *z.shape[1:]), z.dtype), sharding
            )
            for z in zero_outs
        ]
        out_arrs = jitted(*concat_in, *zeros)
        return [
            {
                nm: np.asarray(out_arrs[i]).reshape(n_cores, *out_avals[i].shape)[c]
                for i, nm in enumerate(out_names)
            }
            for c in range(n_cores)
        ]

    _CACHE["runner"] = runner
    _CACHE["nc"] = nc
    return runner


def kernel(**inputs) -> np.ndarray:
    runner = get_runner()
    in_maps = make_in_maps(inputs)
    results = runner(in_maps)
    out = np.concatenate(
        [results[c]["y"].reshape(B_LOC, T, C) for c in range(N_CORES)], axis=0
    )
    return out.astype(np.float32)
